# revision 7
# baseline (speedup 1.0000x reference)
"""Batch-parallel attention kernel for 8 TRN2 NeuronCores.

Problem: B=16, S=2048, D=128 full (non-causal) attention, fp32 I/O.
Sharding: batch dim across 8 cores (2 batches/core), no collectives.

Per-core structure (all in "transposed score" space, no on-device
transposes):
  - matmul1: S^T[k,q] = (K^T)[d,k]-stationary.T @ (Q^T)[d,q]-moving,
    contraction over d=128 partitions. Q^T/K^T prepared on host (bf16).
    Outputs grouped 3 k-tiles deep in PSUM so exp runs as few large
    ScalarE calls as possible (ACT is the fundamental bottleneck:
    1 elem/cycle/lane plus ~300-400ns fixed cost per instruction).
  - ScalarE: expS^T = exp(scale * S^T), PSUM->SBUF bf16. No max
    subtraction (scores ~N(0,1), max ~7.5 -> exp(7.5) fine in f32).
  - matmul2: out^T[d,q] += V(kt)[k,d]-stationary.T @ expS^T(kt)[k,q],
    N=512 moving, one PSUM-bank accumulator per 512-q pass. V stays
    stationary per k-tile (no per-q-tile weight reloads).
  - Denominator: VectorE tree-sums the expS^T tiles over k-tiles (bf16
    2x mode), a [128,1]-ones matmul collapses partitions to [1,q],
    DMA broadcasts it back to 128 partitions, VectorE reciprocal +
    tensor_tensor multiply normalizes the accumulated out^T.
  - Output is [d, q]; host transposes back to [q, d] per batch.

q is processed in 4 passes of 512 per batch; k in 6 exp-groups
(3+3+3+3+3+1 k-tiles) per pass. PSUM budget: 2x3 banks m1 groups +
1 bank out^T accumulator + 1 bank denominator row = 8.
"""

import os

import ml_dtypes
import numpy as np

import concourse.bass as bass
import concourse.mybir as mybir
import concourse.tile as tile
from concourse import bacc
from concourse.bass_utils import run_bass_kernel_spmd

B, S, D = 16, 2048, 128
N_CORES = 8
BPC = B // N_CORES          # batches per core
QP = 512                    # q per pass
N_PASS = S // QP            # 4
N_KT = S // 128             # 16 k-tiles
KT_GROUPS = [(0, 3), (3, 3), (6, 3), (9, 3), (12, 3), (15, 1)]
SCALE = 1.0 / float(np.sqrt(D))

BF16 = mybir.dt.bfloat16
F32 = mybir.dt.float32

TRACE = bool(os.environ.get("BASS_KERNEL_TRACE"))
LAST_RESULTS = None

_CACHE = {}


def _build():
    nc = bacc.Bacc("TRN2", target_bir_lowering=False, debug=False)

    qT = nc.dram_tensor("qT", [BPC, D, S], BF16, kind="ExternalInput").ap()
    kT = nc.dram_tensor("kT", [BPC, D, S], BF16, kind="ExternalInput").ap()
    vD = nc.dram_tensor("vD", [BPC, S, D], BF16, kind="ExternalInput").ap()
    outT = nc.dram_tensor("outT", [BPC, D, S], F32, kind="ExternalOutput").ap()

    with tile.TileContext(nc) as tc:
        with (
            tc.tile_pool(name="qk", bufs=2) as qk_pool,
            tc.tile_pool(name="vp", bufs=2) as v_pool,
            tc.tile_pool(name="ones", bufs=1) as ones_pool,
            tc.tile_pool(name="pexp", bufs=3) as p_pool,
            tc.tile_pool(name="dvec", bufs=2) as d_pool,
            tc.tile_pool(name="outs", bufs=4) as o_pool,
            tc.tile_pool(name="psum_s", bufs=2, space="PSUM") as psum_s,
            tc.tile_pool(name="psum_o", bufs=1, space="PSUM") as psum_o,
            tc.tile_pool(name="psum_d", bufs=1, space="PSUM") as psum_d,
        ):
            ones_sb = ones_pool.tile([128, 1], BF16)
            nc.vector.memset(ones_sb, 1.0)

            for b in range(BPC):
                qT_sb = qk_pool.tile([128, S], BF16, tag="qT")
                kT_sb = qk_pool.tile([128, S], BF16, tag="kT")
                v_sb = v_pool.tile([128, N_KT, D], BF16)
                nc.sync.dma_start(out=qT_sb, in_=qT[b])
                nc.sync.dma_start(out=kT_sb, in_=kT[b])
                nc.sync.dma_start(
                    out=v_sb, in_=vD[b].rearrange("(t p) d -> p t d", p=128)
                )

                for qc in range(N_PASS):
                    qs = qc * QP
                    acc = psum_o.tile([128, QP], F32)

                    # m2 for group (gi) is emitted after m1 of group (gi+1)
                    # so PE always has independent work while ACT runs exp.
                    pending = None
                    p_tiles = []

                    def emit_m2(kt0, n_kt, p_tile):
                        for h in range(n_kt):
                            kt = kt0 + h
                            nc.tensor.matmul(
                                acc,
                                lhsT=v_sb[:, kt, :],
                                rhs=p_tile[:, h, :],
                                start=(kt == 0),
                                stop=(kt == N_KT - 1),
                            )

                    for kt0, n_kt in KT_GROUPS:
                        s_psum = psum_s.tile(
                            [128, 3, QP], F32, tag="s", name="s_psum"
                        )
                        for h in range(n_kt):
                            nc.tensor.matmul(
                                s_psum[:, h, :],
                                lhsT=kT_sb[:, (kt0 + h) * 128 : (kt0 + h + 1) * 128],
                                rhs=qT_sb[:, qs : qs + QP],
                                start=True,
                                stop=True,
                            )
                        p_tile = p_pool.tile([128, 3, QP], BF16, tag="p", name="p_tile")
                        nc.scalar.activation(
                            p_tile[:, 0:n_kt, :],
                            s_psum[:, 0:n_kt, :],
                            mybir.ActivationFunctionType.Exp,
                            scale=SCALE,
                        )
                        p_tiles.append((n_kt, p_tile))
                        if pending is not None:
                            emit_m2(*pending)
                        pending = (kt0, n_kt, p_tile)
                    emit_m2(*pending)

                    # denominator: tree-sum the 6 exp tiles over k
                    # (elementwise over [kt_sub, q] is valid: every k lands
                    # in the total eventually), then collapse partitions.
                    g = [t for _, t in p_tiles]
                    t01 = d_pool.tile([128, 3, QP], BF16, tag="t01")
                    t23 = d_pool.tile([128, 3, QP], BF16, tag="t23")
                    nc.vector.tensor_add(t01, g[0], g[1])
                    nc.vector.tensor_add(t23, g[2], g[3])
                    nc.vector.tensor_add(t01, t01, t23)
                    nc.vector.tensor_add(t01, t01, g[4])
                    dsum = d_pool.tile([128, QP], BF16, tag="dsum")
                    nc.vector.tensor_add(dsum, t01[:, 0, :], t01[:, 1, :])
                    nc.vector.tensor_add(dsum, dsum, t01[:, 2, :])
                    nc.vector.tensor_add(dsum, dsum, g[5][:, 0, :])

                    denom = psum_d.tile([1, QP], F32)
                    nc.tensor.matmul(denom, lhsT=ones_sb, rhs=dsum,
                                     start=True, stop=True)

                    recip_row = o_pool.tile([1, QP], F32, tag="recip_row")
                    nc.vector.reciprocal(recip_row, denom)
                    recip = o_pool.tile([128, QP], F32, tag="recip")
                    nc.gpsimd.partition_broadcast(recip, recip_row)
                    o_sb = o_pool.tile([128, QP], F32, tag="o")
                    nc.vector.tensor_mul(o_sb, acc, recip)
                    nc.sync.dma_start(out=outT[b, :, qs : qs + QP], in_=o_sb)

    nc.compile()
    return nc


def _get_nc():
    if "nc" not in _CACHE:
        _CACHE["nc"] = _build()
    return _CACHE["nc"]


def kernel(query, key, value):
    global LAST_RESULTS
    bf16 = ml_dtypes.bfloat16
    q = np.ascontiguousarray(
        np.asarray(query, dtype=np.float32).transpose(0, 2, 1)
    ).astype(bf16)
    k = np.ascontiguousarray(
        np.asarray(key, dtype=np.float32).transpose(0, 2, 1)
    ).astype(bf16)
    v = np.asarray(value, dtype=np.float32).astype(bf16)

    nc = _get_nc()
    in_maps = [
        {
            "qT": q[i * BPC : (i + 1) * BPC],
            "kT": k[i * BPC : (i + 1) * BPC],
            "vD": v[i * BPC : (i + 1) * BPC],
        }
        for i in range(N_CORES)
    ]
    res = run_bass_kernel_spmd(
        nc, in_maps, core_ids=list(range(N_CORES)), trace=TRACE
    )
    LAST_RESULTS = res
    out = np.empty((B, S, D), dtype=np.float32)
    for i in range(N_CORES):
        o = res.results[i]["outT"]  # [BPC, D, S]
        out[i * BPC : (i + 1) * BPC] = o.transpose(0, 2, 1)
    return out


# revision 10
# speedup vs baseline: 1.5489x; 1.5489x over previous
"""Batch-parallel attention kernel for 8 TRN2 NeuronCores.

Problem: B=16, S=2048, D=128 full (non-causal) attention, fp32 I/O.
Sharding: batch dim across 8 cores (2 batches/core), no collectives.

Per-core layout trick: everything is computed in "transposed score" space
S^T[k, q] so that no on-device transposes are needed:
  - matmul1: S^T[k,q] = (K^T)[d,k]^T-stationary @ (Q^T)[d,q]-moving,
    contraction over d=128 partitions. Q^T/K^T are prepared on host.
  - ScalarE: expS^T = exp(scale * S^T) PSUM->SBUF (bf16), no max
    subtraction (scores are ~N(0,1); max over dataset ~7.5 -> exp fine).
  - matmul2: out[q, 0:129] = sum_k expS^T[k,q]^T-stationary @ V_aug[k,:]
    where V_aug = [V | ones]; column 128 accumulates the softmax
    denominator exactly in fp32 PSUM.
  - VectorE: reciprocal of the denominator column + per-partition
    tensor_scalar multiply -> normalized out tile, DMA'd out natively.
"""

import os

import ml_dtypes
import numpy as np

import concourse.bass as bass
import concourse.mybir as mybir
import concourse.tile as tile
from concourse import bacc
from concourse.bass_utils import run_bass_kernel_spmd

B, S, D = 16, 2048, 128
N_CORES = 8
BPC = B // N_CORES          # batches per core
DA = D + 1                  # V augmented with ones column
QCHUNK = 512                # q processed per inner pipeline chunk
N_QC = S // QCHUNK          # 4
N_KT = S // 128             # 16 k-tiles
SCALE = 1.0 / float(np.sqrt(D))

BF16 = mybir.dt.bfloat16
F32 = mybir.dt.float32

TRACE = bool(os.environ.get("BASS_KERNEL_TRACE"))
LAST_RESULTS = None

_CACHE = {}


def _build():
    nc = bacc.Bacc("TRN2", target_bir_lowering=False, debug=False)

    qT = nc.dram_tensor("qT", [BPC, D, S], BF16, kind="ExternalInput").ap()
    kT = nc.dram_tensor("kT", [BPC, D, S], BF16, kind="ExternalInput").ap()
    vA = nc.dram_tensor("vA", [BPC, S, DA], BF16, kind="ExternalInput").ap()
    out = nc.dram_tensor("out", [BPC, S, D], F32, kind="ExternalOutput").ap()

    with tile.TileContext(nc) as tc:
        with (
            tc.tile_pool(name="qk", bufs=2) as qk_pool,
            tc.tile_pool(name="vp", bufs=2) as v_pool,
            tc.tile_pool(name="warm", bufs=1) as warm_pool,
            tc.tile_pool(name="pexp", bufs=4) as p_pool,
            tc.tile_pool(name="outs", bufs=8) as o_pool,
            tc.tile_pool(name="psum_s", bufs=2, space="PSUM") as psum_s,
            tc.tile_pool(name="psum_acc", bufs=1, space="PSUM") as psum_acc,
        ):
            # Pull the ~2.7us exp table load to t=0 so it overlaps the input
            # DMAs instead of stalling the first real exp.
            wtile = warm_pool.tile([128, 1], F32)
            nc.vector.memset(wtile, 0.0)
            nc.scalar.activation(
                wtile, wtile, mybir.ActivationFunctionType.Exp
            )

            for b in range(BPC):
                qT_sb = qk_pool.tile([128, S], BF16, tag="qT")
                kT_sb = qk_pool.tile([128, S], BF16, tag="kT")
                v_sb = v_pool.tile([128, N_KT, DA], BF16)
                # Spread the loads over three engines' DMA queues so they
                # run in parallel; halves let the first m1 start sooner.
                H = S // 2
                nc.sync.dma_start(out=kT_sb[:, 0:H], in_=kT[b][:, 0:H])
                nc.sync.dma_start(out=qT_sb[:, 0:H], in_=qT[b][:, 0:H])
                nc.scalar.dma_start(out=kT_sb[:, H:S], in_=kT[b][:, H:S])
                nc.scalar.dma_start(out=qT_sb[:, H:S], in_=qT[b][:, H:S])
                nc.gpsimd.dma_start(
                    out=v_sb, in_=vA[b].rearrange("(t p) d -> p t d", p=128)
                )

                for qc in range(N_QC):
                    qs = qc * QCHUNK
                    acc = [
                        psum_acc.tile(
                            [128, DA], F32, tag=f"acc{j}", name=f"acc{j}"
                        )
                        for j in range(4)
                    ]

                    # software-pipelined with a 2-deep skew: m2 for kt-pair g
                    # is emitted after m1 of pair g+2, so the in-order PE
                    # queue always has independent m1 work while exp runs and
                    # while the previous q-chunk's accumulators drain.
                    pending = []  # [(kt0, p_tile), ...] awaiting matmul2

                    def emit_m2(kt0, p_tile):
                        for h in range(2):
                            kt = kt0 + h
                            for j in range(4):
                                nc.tensor.matmul(
                                    acc[j],
                                    lhsT=p_tile[:, h, j * 128 : (j + 1) * 128],
                                    rhs=v_sb[:, kt, :],
                                    start=(kt == 0),
                                    stop=(kt == N_KT - 1),
                                )

                    for kt0 in range(0, N_KT, 2):
                        s_psum = psum_s.tile([128, 2, QCHUNK], F32)
                        for h in range(2):
                            nc.tensor.matmul(
                                s_psum[:, h, :],
                                lhsT=kT_sb[:, (kt0 + h) * 128 : (kt0 + h + 1) * 128],
                                rhs=qT_sb[:, qs : qs + QCHUNK],
                                start=True,
                                stop=True,
                            )
                        p_tile = p_pool.tile([128, 2, QCHUNK], BF16)
                        nc.scalar.activation(
                            p_tile,
                            s_psum,
                            mybir.ActivationFunctionType.Exp,
                            scale=SCALE,
                        )
                        pending.append((kt0, p_tile))
                        if len(pending) > 2:
                            emit_m2(*pending.pop(0))
                    for args in pending:
                        emit_m2(*args)

                    for j in range(4):
                        recip = o_pool.tile([128, 1], F32, tag="recip")
                        nc.vector.reciprocal(recip, acc[j][:, D : D + 1])
                        o_sb = o_pool.tile([128, D], F32, tag="o")
                        nc.vector.tensor_scalar_mul(o_sb, acc[j][:, 0:D], recip)
                        r0 = qs + j * 128
                        nc.sync.dma_start(out=out[b, r0 : r0 + 128, :], in_=o_sb)

    nc.compile()
    return nc


def _get_nc():
    if "nc" not in _CACHE:
        _CACHE["nc"] = _build()
    return _CACHE["nc"]


def kernel(query, key, value):
    global LAST_RESULTS
    bf16 = ml_dtypes.bfloat16
    q = np.ascontiguousarray(
        np.asarray(query, dtype=np.float32).transpose(0, 2, 1)
    ).astype(bf16)
    k = np.ascontiguousarray(
        np.asarray(key, dtype=np.float32).transpose(0, 2, 1)
    ).astype(bf16)
    v = np.asarray(value, dtype=np.float32)
    v_aug = np.concatenate(
        [v, np.ones((B, S, 1), dtype=np.float32)], axis=2
    ).astype(bf16)

    nc = _get_nc()
    in_maps = [
        {
            "qT": q[i * BPC : (i + 1) * BPC],
            "kT": k[i * BPC : (i + 1) * BPC],
            "vA": v_aug[i * BPC : (i + 1) * BPC],
        }
        for i in range(N_CORES)
    ]
    res = run_bass_kernel_spmd(
        nc, in_maps, core_ids=list(range(N_CORES)), trace=TRACE
    )
    LAST_RESULTS = res
    out = np.empty((B, S, D), dtype=np.float32)
    for i in range(N_CORES):
        out[i * BPC : (i + 1) * BPC] = res.results[i]["out"]
    return out
